# revision 20
# baseline (speedup 1.0000x reference)
"""Trainium2 Bass kernel for the RNN greedy-decoder (topk_masking) problem.

Full-input contract: kernel(**inputs) takes the complete unsharded inputs and
returns (loss, tours) exactly like the reference. Internally the batch (256)
is sharded 8 ways across NeuronCores (32 rows/core); the tiny RNN/linear
params are replicated; the loss reduction is finished on host.

Device decomposition per core (32 batch rows, 256 steps):
- PE: RNN recurrence (W_hh matmul + rank-1 target embedding), logits matmul
  with b_out added via identity-matmul PSUM accumulation. PE has no mask
  dependency, so it runs arbitrarily far ahead of the decode chain.
- VE: the entire sequential decode chain, with no cross-engine hops:
  mask-apply (maskcount*-BIG + logits, PSUM->SBUF, written into one third of
  a [96,256] stacked tile), argmax (max8 + max_index), index convert, and
  the mask update (maskcount += (iota == chosen)) as one fused op each.
- ACT: tanh (fused per-partition bias); softmax exp with accum_out (sum);
  second exp for the reference's log_softmax(softmax(x)) quirk -- all
  batched over 3-step partition-stacked groups.
- Host: final loss reduction and output assembly.
"""

import numpy as np

B_LOC = 32  # batch rows per core
T = 256
H = 128
NCORES = 8
NG = (T + 2) // 3  # 3-step stacked groups (matmul out base must be 0/32/64)
BIG = 1.0e30

_CACHE = {}


def _build_nc():
    import concourse.bass as bass
    import concourse.tile as tile
    from concourse import bacc, mybir

    f32 = mybir.dt.float32
    bf16 = mybir.dt.bfloat16
    u16 = mybir.dt.uint16
    AF = mybir.ActivationFunctionType
    OP = mybir.AluOpType

    nc = bacc.Bacc()

    # ---- DRAM I/O (per core) ----
    d_hT0 = nc.dram_tensor("hT0", [H, B_LOC], f32, kind="ExternalInput")
    d_tgtT = nc.dram_tensor("tgtT", [1, T * B_LOC], f32, kind="ExternalInput")
    d_tgt3 = nc.dram_tensor("tgt3", [96, NG], f32, kind="ExternalInput")
    d_WhhT = nc.dram_tensor("WhhT", [H, H], f32, kind="ExternalInput")
    d_WoutT = nc.dram_tensor("WoutT", [H, T], f32, kind="ExternalInput")
    d_urow = nc.dram_tensor("urow", [1, H], f32, kind="ExternalInput")
    d_bias0 = nc.dram_tensor("bias0", [H, 1], f32, kind="ExternalInput")
    d_biasr = nc.dram_tensor("biasr", [H, 1], f32, kind="ExternalInput")
    d_identf = nc.dram_tensor("identf", [B_LOC, B_LOC], f32, kind="ExternalInput")
    d_iotaf = nc.dram_tensor("iotaf", [B_LOC, T], f32, kind="ExternalInput")
    d_iota3 = nc.dram_tensor("iota3", [96, T], f32, kind="ExternalInput")
    d_mask0 = nc.dram_tensor("mask0", [B_LOC, T], bf16, kind="ExternalInput")
    d_boutbc = nc.dram_tensor("boutbc", [B_LOC, T], f32, kind="ExternalInput")
    d_masked0 = nc.dram_tensor("masked0", [B_LOC, T], f32, kind="ExternalInput")

    d_chosen = nc.dram_tensor("chosen", [B_LOC, T], f32, kind="ExternalOutput")
    d_etgt = nc.dram_tensor("etgt", [96, NG], f32, kind="ExternalOutput")
    d_slog = nc.dram_tensor("slog", [96, NG], f32, kind="ExternalOutput")
    d_s2log = nc.dram_tensor("s2log", [96, NG], f32, kind="ExternalOutput")
    d_rlog = nc.dram_tensor("rlog", [96, NG], f32, kind="ExternalOutput")

    with tile.TileContext(nc) as tc:
        with (
            tc.tile_pool(name="persist", bufs=1) as P,
            tc.tile_pool(name="hpool", bufs=3) as HP,
            tc.tile_pool(name="epool", bufs=2) as EP,
            tc.tile_pool(name="small", bufs=4) as SP,
            tc.tile_pool(name="psum_h", bufs=2, space="PSUM") as PH,
            tc.tile_pool(name="psum_l", bufs=4, space="PSUM") as PL,
        ):
            WhhT = P.tile([H, H], f32)
            nc.sync.dma_start(out=WhhT, in_=d_WhhT[:])
            WoutT = P.tile([H, T], f32)
            nc.sync.dma_start(out=WoutT, in_=d_WoutT[:])
            urow = P.tile([1, H], f32)
            nc.sync.dma_start(out=urow, in_=d_urow[:])
            tgtT = P.tile([1, T * B_LOC], f32)
            nc.sync.dma_start(out=tgtT, in_=d_tgtT[:])
            tgt3 = P.tile([96, NG], f32)
            nc.sync.dma_start(out=tgt3, in_=d_tgt3[:])
            bias0 = P.tile([H, 1], f32)
            nc.sync.dma_start(out=bias0, in_=d_bias0[:])
            biasr = P.tile([H, 1], f32)
            nc.sync.dma_start(out=biasr, in_=d_biasr[:])
            identf = P.tile([B_LOC, B_LOC], f32)
            nc.sync.dma_start(out=identf, in_=d_identf[:])
            iotaf = P.tile([B_LOC, T], f32)
            nc.sync.dma_start(out=iotaf, in_=d_iotaf[:])
            iota3 = P.tile([96, T], f32)
            nc.sync.dma_start(out=iota3, in_=d_iota3[:])
            maskv = P.tile([B_LOC, T], bf16)
            nc.sync.dma_start(out=maskv, in_=d_mask0[:])
            boutbc = P.tile([B_LOC, T], f32)
            nc.sync.dma_start(out=boutbc, in_=d_boutbc[:])
            masked0 = P.tile([B_LOC, T], f32)
            nc.sync.dma_start(out=masked0, in_=d_masked0[:])
            hT0 = P.tile([H, B_LOC], f32)
            nc.sync.dma_start(out=hT0, in_=d_hT0[:])

            chosen = P.tile([B_LOC, T], f32)
            etgt = P.tile([96, NG], f32)
            slog = P.tile([96, NG], f32)
            s2log = P.tile([96, NG], f32)
            rlog = P.tile([96, NG], f32)

            nc.vector.memset(chosen[:, 0:1], 0.0)

            hT_prev = hT0
            for g in range(NG):
                pl3 = PL.tile([96, T], f32, tag="pl3")
                msb3 = EP.tile([96, T], f32, tag="msb3")
                for s in range(3):
                    t = 3 * g + s
                    q = pl3[32 * s : 32 * s + 32, :]
                    m = msb3[32 * s : 32 * s + 32, :]
                    if t > T - 1:
                        # filler so the group CE reads defined data
                        nc.vector.tensor_copy(m, masked0)
                        continue

                    # recurrence: h_t = tanh(W_hh h_{t-1} [+ u*tgt_{t-1}] + bias)
                    ph = PH.tile([H, B_LOC], f32, tag="ph")
                    if t == 0:
                        nc.tensor.matmul(ph, lhsT=WhhT, rhs=hT_prev, start=True, stop=True)
                    else:
                        nc.tensor.matmul(ph, lhsT=WhhT, rhs=hT_prev, start=True, stop=False)
                        nc.tensor.matmul(
                            ph, lhsT=urow, rhs=tgtT[:, (t - 1) * B_LOC : t * B_LOC],
                            start=False, stop=True,
                        )
                    hT_cur = HP.tile([H, B_LOC], f32, tag="hT")
                    nc.scalar.activation(
                        hT_cur, ph, AF.Tanh, bias=(bias0 if t == 0 else biasr), scale=1.0
                    )

                    if t == 0:
                        # forced first step: masked_0 is a host constant
                        nc.vector.tensor_copy(m, masked0)
                    else:
                        # logits + b_out in PSUM (PE runs ahead; no mask dep)
                        nc.tensor.matmul(q, lhsT=hT_cur, rhs=WoutT, start=True, stop=False)
                        nc.tensor.matmul(q, lhsT=identf, rhs=boutbc, start=False, stop=True)

                        # VE-local chain: apply mask, argmax, update mask
                        nc.vector.scalar_tensor_tensor(
                            out=m, in0=maskv, scalar=-BIG, in1=q,
                            op0=OP.mult, op1=OP.add,
                        )
                        rep = SP.tile([B_LOC, 8], f32, tag="rep")
                        nc.vector.max(rep, m)
                        idx8 = SP.tile([B_LOC, 8], u16, tag="idx8")
                        nc.vector.max_index(idx8, rep, m)
                        nc.vector.tensor_copy(chosen[:, t : t + 1], idx8[:, 0:1])
                        nc.vector.scalar_tensor_tensor(
                            out=maskv, in0=iotaf, scalar=chosen[:, t : t + 1],
                            in1=maskv, op0=OP.is_equal, op1=OP.add,
                        )

                    hT_prev = hT_cur

                # CE for the 3-step group, partition-stacked [96, 256]
                e3 = EP.tile([96, T], f32, tag="e3")
                nc.scalar.activation(
                    e3, msb3, AF.Exp, bias=0.0, scale=1.0,
                    accum_out=slog[:, g : g + 1],
                )
                nc.vector.reciprocal(rlog[:, g : g + 1], slog[:, g : g + 1])
                e2j = EP.tile([96, T], f32, tag="e2j")
                nc.scalar.activation(
                    e2j, e3, AF.Exp, bias=0.0, scale=rlog[:, g : g + 1],
                    accum_out=s2log[:, g : g + 1],
                )
                gj = EP.tile([96, T], f32, tag="gj")
                nc.vector.scalar_tensor_tensor(
                    out=gj, in0=iota3, scalar=tgt3[:, g : g + 1], in1=e3,
                    op0=OP.is_equal, op1=OP.mult, accum_out=etgt[:, g : g + 1],
                )

            nc.sync.dma_start(out=d_chosen[:], in_=chosen)
            nc.sync.dma_start(out=d_etgt[:], in_=etgt)
            nc.sync.dma_start(out=d_slog[:], in_=slog)
            nc.sync.dma_start(out=d_s2log[:], in_=s2log)
            nc.sync.dma_start(out=d_rlog[:], in_=rlog)

    nc.finalize()
    return nc


def _host_prep(inputs):
    """Build the per-core input maps (host-side layout prep only)."""
    import ml_dtypes

    bf16 = ml_dtypes.bfloat16
    enc = np.asarray(inputs["encoder_context"], np.float32)
    tg = np.asarray(inputs["targets"])
    W_ih = np.asarray(inputs["W_ih"], np.float32)
    W_hh = np.asarray(inputs["W_hh"], np.float32)
    b_ih = np.asarray(inputs["b_ih"], np.float32)
    b_hh = np.asarray(inputs["b_hh"], np.float32)
    W_out = np.asarray(inputs["W_out"], np.float32)
    b_out = np.asarray(inputs["b_out"], np.float32)
    W_in = np.asarray(inputs["W_in"], np.float32)
    b_in = np.asarray(inputs["b_in"], np.float32)

    # fold dec_in = tgt*w + b_in through W_ih: pre = tgt*u + c
    u = (W_ih.astype(np.float64) @ W_in[:, 0].astype(np.float64)).astype(np.float32)
    c = (W_ih.astype(np.float64) @ b_in.astype(np.float64)).astype(np.float32) + b_ih
    c0 = W_ih.astype(np.float64).sum(axis=1).astype(np.float32) + b_ih  # ones @ W_ih.T

    WhhT = np.ascontiguousarray(W_hh.T)
    WoutT = np.ascontiguousarray(W_out.T)
    bias0 = (c0 + b_hh).reshape(H, 1)
    biasr = (c + b_hh).reshape(H, 1)
    identf = np.eye(B_LOC, dtype=np.float32)
    iotaf = np.broadcast_to(np.arange(T, dtype=np.float32), (B_LOC, T)).copy()
    iota3 = np.broadcast_to(np.arange(T, dtype=np.float32), (96, T)).copy()
    mask0 = np.zeros((B_LOC, T), bf16)
    mask0[:, 0] = 1.0  # chosen_0 == 0 is masked from t=1 on
    boutbc = np.broadcast_to(b_out, (B_LOC, T)).astype(np.float32).copy()
    masked0 = np.full((B_LOC, T), -BIG, np.float32)
    masked0[:, 0] = 1.0

    in_maps = []
    for core in range(NCORES):
        b0 = core * B_LOC
        tgf = tg[b0 : b0 + B_LOC, :].astype(np.float32)  # [32, 256]
        # tgt3[32*s + b, g] = targets[b, 3*g + s]
        tgt3 = np.zeros((96, NG), np.float32)
        for s in range(3):
            ts_idx = np.arange(s, T, 3)
            tgt3[32 * s : 32 * s + 32, : len(ts_idx)] = tgf[:, ts_idx]
        in_maps.append(
            {
                "hT0": np.ascontiguousarray(enc[0, b0 : b0 + B_LOC, :].T),
                "tgtT": np.ascontiguousarray(tgf.T).reshape(1, T * B_LOC),
                "tgt3": tgt3,
                "WhhT": WhhT,
                "WoutT": WoutT,
                "urow": u.reshape(1, H),
                "bias0": bias0,
                "biasr": biasr,
                "identf": identf,
                    "iotaf": iotaf,
                "iota3": iota3,
                "mask0": mask0,
                "boutbc": boutbc,
                "masked0": masked0,
            }
        )
    return in_maps


def _unstack(a3):
    """[96, NG] stacked -> [32, 256]: a[b, 3g+s] = a3[32s+b, g]."""
    out = np.empty((B_LOC, T), a3.dtype)
    for s in range(3):
        ts_idx = np.arange(s, T, 3)
        out[:, ts_idx] = a3[32 * s : 32 * s + 32, : len(ts_idx)]
    return out


def _finish(results):
    """Gather per-core outputs into (loss, tours)."""
    B = NCORES * B_LOC
    chosen = np.concatenate([r["chosen"] for r in results], axis=0)  # [B, T]
    etgt = np.concatenate([_unstack(r["etgt"]) for r in results], axis=0)
    s2log = np.concatenate([_unstack(r["s2log"]) for r in results], axis=0)
    rlog = np.concatenate([_unstack(r["rlog"]) for r in results], axis=0)

    sm_tgt = (etgt * rlog).astype(np.float32)
    lse2 = np.log(s2log).astype(np.float32)
    loss = np.float32(
        -(sm_tgt.astype(np.float64) - lse2.astype(np.float64)).sum() / (B * B)
    )
    tours = chosen.astype(np.float32).reshape(B, 1, T)
    return loss, tours


def kernel(**inputs):
    from concourse.bass_utils import run_bass_kernel_spmd

    if "nc" not in _CACHE:
        _CACHE["nc"] = _build_nc()
    nc = _CACHE["nc"]
    in_maps = _host_prep(inputs)
    res = run_bass_kernel_spmd(nc, in_maps, list(range(NCORES)))
    return _finish(res.results)
